# revision 6
# baseline (speedup 1.0000x reference)
"""Trainium2 Bass kernel for nn_Decoder (LSTM decoder: embed -> LSTM -> vocab fc).

Sharding: 8-way tensor-parallel over the 4H gate dim (and vocab for fc).
Each core computes ALL 128 batch rows x its 512 gate columns (128 of each
of i,f,o,g), so every matmul runs with a full 128-wide stationary operand.
Per step the core's h^T chunk (PE-transposed) is AllGathered via a DRAM
bounce; the 8 gathered chunks land in one fused hsT tile with a single
strided DMA.  Embedding rows are gathered in a prologue and staged to
DRAM; xs^T chunks stream back through transposing DMAs, feeding an
in-loop xg GEMM a few steps ahead.  fc (vocab-sharded, 10x128 rows per
core) is interleaved into the PE/Act idle time under the gather latency.
"""

import sys

sys.path.insert(0, "/opt/trn_rl_repo")

import numpy as np
import ml_dtypes

import concourse.bass as bass
import concourse.bacc as bacc
import concourse.mybir as mybir
import concourse.tile as tile

BF16 = ml_dtypes.bfloat16

# Problem shapes
B, T, E, H, V = 128, 64, 512, 1024, 10000
NCORES = 8
GS = 512            # gate cols per core (128 of each of i,f,o,g)
HC = 128            # h cols per core
NVT = 10            # vocab tiles of 128 per core (80 total, V padded 10240)
VPAD = NCORES * NVT * 128
NTOK = T * B        # 8192 tokens
LOOKAHEAD = 4       # xg compute-ahead distance (steps)
FC_BLOCK = 4        # steps per fc token block (512 tokens)

F32 = mybir.dt.float32
BF = mybir.dt.bfloat16
I32 = mybir.dt.int32


def build_nc(t_steps=T):
    nc = bacc.Bacc(num_devices=NCORES)

    embed_d = nc.declare_dram_parameter("embed", [V + B, E], BF, isOutput=False)
    idx_d = nc.declare_dram_parameter("idx", [B, t_steps], I32, isOutput=False)
    # wmov: k 0..3 = W_ih^T chunks, k 4 = bias row, k 5..12 = W_hh^T chunks
    wmov_d = nc.declare_dram_parameter("wmov", [13, 128, GS], BF, isOutput=False)
    eye_d = nc.declare_dram_parameter("eye128", [128, 128], BF, isOutput=False)
    ones_d = nc.declare_dram_parameter("onesrow", [128, 128], BF, isOutput=False)
    fcw_d = nc.declare_dram_parameter("fcw", [NVT, 128, 8 * 128], BF, isOutput=False)
    fcb_d = nc.declare_dram_parameter("fcb", [128, NVT], F32, isOutput=False)
    out_d = nc.declare_dram_parameter("out_lg", [NVT, 128, t_steps * B], F32,
                                      isOutput=True)
    xsr_d = nc.dram_tensor("xsr_stage", [t_steps, B, E], BF)

    blk_tok = FC_BLOCK * B

    with tile.TileContext(nc) as tc:
        with (
            tc.tile_pool(name="persist", bufs=1) as pp,
            tc.tile_pool(name="xsraw", bufs=6) as xsraw_p,
            tc.tile_pool(name="xst", bufs=3) as xst_p,
            tc.tile_pool(name="xg", bufs=LOOKAHEAD + 2) as xg_p,
            tc.tile_pool(name="gates", bufs=3) as gates_p,
            tc.tile_pool(name="ew", bufs=4) as ew_p,
            tc.tile_pool(name="ht", bufs=3) as ht_p,
            tc.tile_pool(name="logit", bufs=4) as logit_p,
            tc.tile_pool(name="gpsum", bufs=2, space="PSUM") as gps_p,
            tc.tile_pool(name="xpsum", bufs=1, space="PSUM") as xps_p,
            tc.tile_pool(name="tpsum", bufs=2, space="PSUM") as tps_p,
            tc.tile_pool(name="fpsum", bufs=2, space="PSUM") as fps_p,
            tc.tile_pool(name="dram", bufs=4, space="DRAM") as dram_p,
        ):
            # ---- persistent tiles + prologue loads ----
            wsb = {}
            for k in range(13):
                w = pp.tile([128, GS], BF, tag=f"w_{k}", name=f"w_{k}")
                nc.sync.dma_start(out=w[:, :], in_=wmov_d[k, :, :])
                wsb[k] = w
            idx_sb = pp.tile([B, t_steps], I32, tag="idx")
            nc.sync.dma_start(out=idx_sb[:, :], in_=idx_d[:, :])
            eye_sb = pp.tile([128, 128], BF, tag="eye")
            nc.sync.dma_start(out=eye_sb[:, :], in_=eye_d[:, :])
            ones_sb = pp.tile([128, 128], BF, tag="ones")
            nc.sync.dma_start(out=ones_sb[:, :], in_=ones_d[:, :])
            fcw_sb = []
            for v in range(NVT):
                w = pp.tile([128, 8 * 128], BF, tag=f"fcw_{v}", name=f"fcw_{v}")
                nc.sync.dma_start(out=w[:, :], in_=fcw_d[v, :, :])
                fcw_sb.append(w)
            fcb_sb = pp.tile([128, NVT], F32, tag="fcb")
            nc.sync.dma_start(out=fcb_sb[:, :], in_=fcb_d[:, :])

            # fused h^T history: free index = kc*NTOK + t*B + b
            hsT = pp.tile([128, 8 * NTOK], BF, tag="hsT", name="hsT")
            c_sb = pp.tile([128, HC], F32, tag="c_state")

            # collective bounce buffers (DRAM)
            bounce_in = [dram_p.tile([128, 128], BF, name=f"bi_{i}")
                         for i in range(2)]
            bounce_out = [dram_p.tile([8, 128, 128], BF, name=f"bo_{i}")
                          for i in range(2)]

            # ---- prologue: gather embeddings, stage to DRAM ----
            for t in range(t_steps):
                xsr = xsraw_p.tile([B, E], BF, tag="xsr")
                nc.gpsimd.indirect_dma_start(
                    out=xsr[:, :],
                    out_offset=None,
                    in_=embed_d[:, :],
                    in_offset=bass.IndirectOffsetOnAxis(
                        ap=idx_sb[:, t : t + 1], axis=0
                    ),
                )
                nc.sync.dma_start(out=xsr_d[t, :, :], in_=xsr[:, :])

            # ---- xg pipeline: transposing-DMA xs^T chunks -> gemm ----
            xg_tiles = {}

            def make_xg(t):
                xst = xst_p.tile([128, 4 * 128], BF, tag="xst")
                for ec in range(4):
                    nc.sync.dma_start_transpose(
                        out=xst[:, 128 * ec : 128 * ec + 128],
                        in_=xsr_d[t, :, 128 * ec : 128 * ec + 128],
                    )
                xps = xps_p.tile([128, GS], F32, tag="xps")
                for ec in range(4):
                    nc.tensor.matmul(
                        xps[:, :],
                        xst[:, 128 * ec : 128 * ec + 128],
                        wsb[ec][:, :],
                        start=(ec == 0),
                        stop=False,
                    )
                nc.tensor.matmul(
                    xps[:, :], ones_sb[:, :], wsb[4][:, :], start=False, stop=True
                )
                xg = xg_p.tile([B, GS], BF, tag="xg")
                nc.scalar.activation(
                    xg[:, :], xps[:, :], mybir.ActivationFunctionType.Identity
                )
                xg_tiles[t] = xg

            for t in range(LOOKAHEAD):
                make_xg(t)

            # ---- fc emission helper ----
            fc_queue = []  # (block_idx, vtile)

            def emit_fc(n):
                for _ in range(min(n, len(fc_queue))):
                    kblk, v = fc_queue.pop(0)
                    fps = fps_p.tile([128, blk_tok], F32, tag="fps")
                    for kc in range(8):
                        nc.tensor.matmul(
                            fps[:, :],
                            fcw_sb[v][:, 128 * kc : 128 * kc + 128],
                            hsT[:, kc * NTOK + blk_tok * kblk :
                                kc * NTOK + blk_tok * (kblk + 1)],
                            start=(kc == 0),
                            stop=(kc == 7),
                        )
                    lg = logit_p.tile([128, blk_tok], F32, tag="lg")
                    nc.scalar.activation(
                        lg[:, :],
                        fps[:, :],
                        mybir.ActivationFunctionType.Identity,
                        bias=fcb_sb[:, v : v + 1],
                    )
                    nc.sync.dma_start(
                        out=out_d[v, :, blk_tok * kblk : blk_tok * (kblk + 1)],
                        in_=lg[:, :],
                    )

            # ---- recurrence ----
            for t in range(t_steps):
                ps = gps_p.tile([128, GS], F32, tag="gps")
                # xg injection (identity stationary)
                nc.tensor.matmul(
                    ps[:, :], eye_sb[:, :], xg_tiles.pop(t)[:, :],
                    start=True, stop=(t == 0),
                )
                if t > 0:
                    for kc in range(8):
                        nc.tensor.matmul(
                            ps[:, :],
                            hsT[:, kc * NTOK + B * (t - 1) :
                                kc * NTOK + B * t],
                            wsb[5 + kc][:, :],
                            start=False,
                            stop=(kc == 7),
                        )

                gt = gates_p.tile([128, GS], F32, tag="gt")
                nc.scalar.activation(
                    gt[:, 0:384], ps[:, 0:384],
                    mybir.ActivationFunctionType.Sigmoid,
                )
                nc.scalar.activation(
                    gt[:, 384:512], ps[:, 384:512],
                    mybir.ActivationFunctionType.Tanh,
                )
                # c = f*c + i*g ; h = o*tanh(c)
                if t == 0:
                    nc.vector.tensor_mul(
                        c_sb[:, :], gt[:, 0:128], gt[:, 384:512]
                    )
                else:
                    tmp1 = ew_p.tile([128, HC], F32, tag="tmp1")
                    nc.vector.tensor_mul(tmp1[:, :], gt[:, 0:128], gt[:, 384:512])
                    nc.vector.tensor_mul(c_sb[:, :], gt[:, 128:256], c_sb[:, :])
                    nc.vector.tensor_add(c_sb[:, :], c_sb[:, :], tmp1[:, :])
                tcs = ew_p.tile([128, HC], F32, tag="tcs")
                nc.scalar.activation(
                    tcs[:, :], c_sb[:, :], mybir.ActivationFunctionType.Tanh
                )
                h_sb = ew_p.tile([128, HC], BF, tag="h_sb")
                nc.vector.tensor_mul(h_sb[:, :], gt[:, 256:384], tcs[:, :])

                # PE transpose h chunk -> [128 H-sub, 128 batch]
                tps = tps_p.tile([128, 128], BF, tag="tps")
                nc.tensor.transpose(tps[:, :], h_sb[:, :], eye_sb[:, :])
                hT = ht_p.tile([128, 128], BF, tag="hT")
                nc.vector.tensor_copy(hT[:, :], tps[:, :])

                # exchange: AllGather h^T chunks via DRAM bounce
                bi = bounce_in[t % 2]
                bo = bounce_out[t % 2]
                nc.sync.dma_start(bi[:, :], hT[:, :])
                nc.gpsimd.collective_compute(
                    "AllGather",
                    mybir.AluOpType.bypass,
                    replica_groups=[list(range(NCORES))],
                    ins=[bi[:, :].opt()],
                    outs=[bo[:, :, :].opt()],
                )
                # single strided read-back into the fused hsT tile
                hview = hsT[:, :].rearrange("p (kc tok) -> p kc tok", kc=8)
                nc.sync.dma_start(
                    out=hview[:, :, B * t : B * (t + 1)],
                    in_=bo[:, :, :].rearrange("kc p b -> p kc b"),
                )

                # pipelined xg for future step
                if t + LOOKAHEAD < t_steps:
                    make_xg(t + LOOKAHEAD)

                # queue fc work for completed blocks; interleave emission
                if (t + 1) % FC_BLOCK == 0:
                    kblk = (t + 1) // FC_BLOCK - 1
                    for v in range(NVT):
                        fc_queue.append((kblk, v))
                if t >= FC_BLOCK:
                    emit_fc(3)

            emit_fc(len(fc_queue))

    nc.finalize()
    return nc


def prep_host(features, captions, embed_W, W_ih, W_hh, b_ih, b_hh, fc_W, fc_b,
              t_steps=T):
    """Host-side layout prep. Returns (shared dict, per-core lists)."""
    W_cat = np.concatenate([W_ih, W_hh], axis=1)  # [4H, E+H]
    bias = (b_ih + b_hh).astype(np.float32)

    emb = np.concatenate(
        [embed_W.astype(np.float32), features.astype(np.float32)], axis=0
    ).astype(BF16)
    idx = np.zeros((B, t_steps), np.int32)
    idx[:, 0] = V + np.arange(B)
    if t_steps > 1:
        idx[:, 1:] = captions[:, 1:t_steps].astype(np.int32)

    eye = np.eye(128, dtype=np.float32).astype(BF16)
    onesrow = np.zeros((128, 128), np.float32)
    onesrow[0, :] = 1.0
    onesrow = onesrow.astype(BF16)

    fc_W_pad = np.zeros((VPAD, H), np.float32)
    fc_W_pad[:V] = fc_W
    fc_b_pad = np.zeros((VPAD,), np.float32)
    fc_b_pad[:V] = fc_b

    shared = {"embed": emb, "idx": idx, "eye128": eye, "onesrow": onesrow}

    per_core = []
    for c in range(NCORES):
        # gate rows for this core: i,f,o,g each 128 rows at offset 128c
        hc = np.arange(128 * c, 128 * c + 128)
        rows = np.concatenate([hc, H + hc, 3 * H + hc, 2 * H + hc])
        W_sel = W_cat[rows]  # [512, 1536]
        wmov = np.zeros((13, 128, GS), np.float32)
        for k in range(4):
            wmov[k] = W_sel[:, 128 * k : 128 * k + 128].T
        wmov[4, 0, :] = bias[rows]
        for k in range(8):
            wmov[5 + k] = W_sel[:, E + 128 * k : E + 128 * k + 128].T
        wmov = wmov.astype(BF16)

        vrows = slice(NVT * 128 * c, NVT * 128 * (c + 1))
        # stationary layout per (v, kc): [128 K-part, 128 vocab cols]
        fcw = np.zeros((NVT, 128, 8 * 128), np.float32)
        Wv = fc_W_pad[vrows].reshape(NVT, 128, 8, 128)  # [v, vocab, kc, k]
        for v in range(NVT):
            for kc in range(8):
                fcw[v, :, 128 * kc : 128 * kc + 128] = Wv[v, :, kc, :].T
        fcw = fcw.astype(BF16)
        fcb = np.ascontiguousarray(
            fc_b_pad[vrows].reshape(NVT, 128).T
        ).astype(np.float32)
        per_core.append({"wmov": wmov, "fcw": fcw, "fcb": fcb})
    return shared, per_core


_NC_CACHE = {}


def kernel(features, captions, embed_W, W_ih, W_hh, b_ih, b_hh, fc_W, fc_b):
    from concourse.bass_utils import run_bass_kernel_spmd

    features = np.asarray(features)
    captions = np.asarray(captions)
    embed_W = np.asarray(embed_W)
    W_ih = np.asarray(W_ih)
    W_hh = np.asarray(W_hh)
    b_ih = np.asarray(b_ih)
    b_hh = np.asarray(b_hh)
    fc_W = np.asarray(fc_W)
    fc_b = np.asarray(fc_b)

    if "nc" not in _NC_CACHE:
        _NC_CACHE["nc"] = build_nc()
    nc = _NC_CACHE["nc"]

    shared, per_core = prep_host(
        features, captions, embed_W, W_ih, W_hh, b_ih, b_hh, fc_W, fc_b
    )
    in_maps = [{**shared, **pc} for pc in per_core]
    res = run_bass_kernel_spmd(nc, in_maps, list(range(NCORES)))
    _NC_CACHE["last_results"] = res
    _NC_CACHE["last_in_maps"] = in_maps

    out = np.empty((B, T, V), np.float32)
    for c in range(NCORES):
        lg = res.results[c]["out_lg"]  # [NVT, 128, T*B]
        # vocab row = 1280c + 128v + p ; token col = t*B + b
        chunk = lg.reshape(NVT * 128, T, B).transpose(2, 1, 0)  # [B, T, 1280]
        v0 = NVT * 128 * c
        v1 = min(V, v0 + NVT * 128)
        if v1 > v0:
            out[:, :, v0:v1] = chunk[:, :, : v1 - v0]
    return out


# revision 9
# speedup vs baseline: 1.4519x; 1.4519x over previous
"""Trainium2 Bass kernel for nn_Decoder (LSTM decoder: embed -> LSTM -> vocab fc).

Sharding: 8-way tensor-parallel over the 4H gate dim (and vocab for fc).
Each core computes ALL 128 batch rows x its 512 gate columns (128 of each
of i,f,o,g), so every matmul runs with a full 128-wide stationary operand.
Per step the core's h^T chunk (PE-transposed) is AllGathered via a DRAM
bounce; the 8 gathered chunks land in one fused hsT tile with a single
strided DMA.  Embedding rows are gathered in a prologue and staged to
DRAM; xs^T chunks stream back through transposing DMAs, feeding an
in-loop xg GEMM a few steps ahead.  fc (vocab-sharded, 10x128 rows per
core) is interleaved into the PE/Act idle time under the gather latency.
"""

import sys

sys.path.insert(0, "/opt/trn_rl_repo")

import numpy as np
import ml_dtypes

import concourse.bass as bass
import concourse.bacc as bacc
import concourse.mybir as mybir
import concourse.tile as tile

BF16 = ml_dtypes.bfloat16

# Problem shapes
B, T, E, H, V = 128, 64, 512, 1024, 10000
NCORES = 8
GS = 512            # gate cols per core (128 of each of i,f,o,g)
HC = 128            # h cols per core
NVT = 10            # vocab tiles of 128 per core (80 total, V padded 10240)
VPAD = NCORES * NVT * 128
NTOK = T * B        # 8192 tokens
LOOKAHEAD = 4       # xg compute-ahead distance (steps)
FC_BLOCK = 4        # steps per fc token block (512 tokens)

F32 = mybir.dt.float32
BF = mybir.dt.bfloat16
I32 = mybir.dt.int32


def build_nc(t_steps=T):
    nc = bacc.Bacc(num_devices=NCORES)

    embed_d = nc.declare_dram_parameter("embed", [V + B, E], BF, isOutput=False)
    idx_d = nc.declare_dram_parameter("idx", [B, t_steps], I32, isOutput=False)
    # wmov: k 0..3 = W_ih^T chunks, k 4 = bias row, k 5..12 = W_hh^T chunks
    wmov_d = nc.declare_dram_parameter("wmov", [13, 128, GS], BF, isOutput=False)
    eye_d = nc.declare_dram_parameter("eye128", [128, 128], BF, isOutput=False)
    ones_d = nc.declare_dram_parameter("onesrow", [128, 128], BF, isOutput=False)
    fcw_d = nc.declare_dram_parameter("fcw", [NVT, 128, 8 * 128], BF, isOutput=False)
    fcb_d = nc.declare_dram_parameter("fcb", [128, NVT], F32, isOutput=False)
    out_d = nc.declare_dram_parameter("out_lg", [NVT, 128, t_steps * B], F32,
                                      isOutput=True)
    xsr_d = nc.dram_tensor("xsr_stage", [t_steps, B, E], BF)

    blk_tok = FC_BLOCK * B

    with tile.TileContext(nc) as tc:
        with (
            tc.tile_pool(name="persist", bufs=1) as pp,
            tc.tile_pool(name="xsraw", bufs=6) as xsraw_p,
            tc.tile_pool(name="xst", bufs=3) as xst_p,
            tc.tile_pool(name="xg", bufs=LOOKAHEAD + 2) as xg_p,
            tc.tile_pool(name="gates", bufs=3) as gates_p,
            tc.tile_pool(name="ew", bufs=4) as ew_p,
            tc.tile_pool(name="ht", bufs=3) as ht_p,
            tc.tile_pool(name="logit", bufs=4) as logit_p,
            tc.tile_pool(name="gpsum", bufs=2, space="PSUM") as gps_p,
            tc.tile_pool(name="xpsum", bufs=1, space="PSUM") as xps_p,
            tc.tile_pool(name="tpsum", bufs=2, space="PSUM") as tps_p,
            tc.tile_pool(name="fpsum", bufs=3, space="PSUM") as fps_p,
            tc.tile_pool(name="dram", bufs=4, space="DRAM") as dram_p,
        ):
            # ---- persistent tiles + prologue loads ----
            wsb = {}
            for k in range(13):
                w = pp.tile([128, GS], BF, tag=f"w_{k}", name=f"w_{k}")
                nc.sync.dma_start(out=w[:, :], in_=wmov_d[k, :, :])
                wsb[k] = w
            idx_sb = pp.tile([B, t_steps], I32, tag="idx")
            nc.sync.dma_start(out=idx_sb[:, :], in_=idx_d[:, :])
            eye_sb = pp.tile([128, 128], BF, tag="eye")
            nc.sync.dma_start(out=eye_sb[:, :], in_=eye_d[:, :])
            ones_sb = pp.tile([128, 128], BF, tag="ones")
            nc.sync.dma_start(out=ones_sb[:, :], in_=ones_d[:, :])
            fcw_sb = []
            for v in range(NVT):
                w = pp.tile([128, 8 * 128], BF, tag=f"fcw_{v}", name=f"fcw_{v}")
                nc.sync.dma_start(out=w[:, :], in_=fcw_d[v, :, :])
                fcw_sb.append(w)
            fcb_sb = pp.tile([128, NVT], F32, tag="fcb")
            nc.sync.dma_start(out=fcb_sb[:, :], in_=fcb_d[:, :])

            # fused h^T history: free index = kc*NTOK + t*B + b
            hsT = pp.tile([128, 8 * NTOK], BF, tag="hsT", name="hsT")
            c_sb = pp.tile([128, HC], F32, tag="c_state")

            # collective bounce buffers (DRAM)
            bounce_in = [dram_p.tile([128, 128], BF, name=f"bi_{i}")
                         for i in range(2)]
            bounce_out = [dram_p.tile([8, 128, 128], BF, name=f"bo_{i}")
                          for i in range(2)]

            # ---- prologue: gather embeddings, stage to DRAM ----
            for t in range(t_steps):
                xsr = xsraw_p.tile([B, E], BF, tag="xsr")
                nc.gpsimd.indirect_dma_start(
                    out=xsr[:, :],
                    out_offset=None,
                    in_=embed_d[:, :],
                    in_offset=bass.IndirectOffsetOnAxis(
                        ap=idx_sb[:, t : t + 1], axis=0
                    ),
                )
                nc.sync.dma_start(out=xsr_d[t, :, :], in_=xsr[:, :])

            # ---- xg pipeline: transposing-DMA xs^T chunks -> gemm ----
            xg_tiles = {}

            def make_xg(t):
                xst = xst_p.tile([128, 4 * 128], BF, tag="xst")
                for ec in range(4):
                    nc.sync.dma_start_transpose(
                        out=xst[:, 128 * ec : 128 * ec + 128],
                        in_=xsr_d[t, :, 128 * ec : 128 * ec + 128],
                    )
                xps = xps_p.tile([128, GS], F32, tag="xps")
                for ec in range(4):
                    nc.tensor.matmul(
                        xps[:, :],
                        xst[:, 128 * ec : 128 * ec + 128],
                        wsb[ec][:, :],
                        start=(ec == 0),
                        stop=False,
                    )
                nc.tensor.matmul(
                    xps[:, :], ones_sb[:, :], wsb[4][:, :], start=False, stop=True
                )
                xg = xg_p.tile([B, GS], BF, tag="xg")
                nc.scalar.activation(
                    xg[:, :], xps[:, :], mybir.ActivationFunctionType.Identity
                )
                xg_tiles[t] = xg

            for t in range(LOOKAHEAD):
                make_xg(t)

            # ---- fc emission helper ----
            fc_queue = []  # (block_idx, vtile)

            def emit_fc(n):
                for _ in range(min(n, len(fc_queue))):
                    kblk, v = fc_queue.pop(0)
                    fps = fps_p.tile([128, blk_tok], F32, tag="fps")
                    for kc in range(8):
                        nc.tensor.matmul(
                            fps[:, :],
                            fcw_sb[v][:, 128 * kc : 128 * kc + 128],
                            hsT[:, kc * NTOK + blk_tok * kblk :
                                kc * NTOK + blk_tok * (kblk + 1)],
                            start=(kc == 0),
                            stop=(kc == 7),
                        )
                    lg = logit_p.tile([128, blk_tok], F32, tag="lg")
                    nc.scalar.activation(
                        lg[:, :],
                        fps[:, :],
                        mybir.ActivationFunctionType.Identity,
                        bias=fcb_sb[:, v : v + 1],
                    )
                    nc.sync.dma_start(
                        out=out_d[v, :, blk_tok * kblk : blk_tok * (kblk + 1)],
                        in_=lg[:, :],
                    )

            # ---- recurrence ----
            for t in range(t_steps):
                # emit future-xg and fc work FIRST so their DMAs/mms sit
                # ahead of the exchange in every queue and fill the
                # collective-latency window (no head-of-line blocking).
                if t + LOOKAHEAD < t_steps:
                    make_xg(t + LOOKAHEAD)
                if t % FC_BLOCK == 0 and t >= FC_BLOCK:
                    kblk = t // FC_BLOCK - 1
                    for v in range(NVT):
                        fc_queue.append((kblk, v))
                if t >= FC_BLOCK:
                    emit_fc(3)

                ps = gps_p.tile([128, GS], F32, tag="gps")
                # xg injection (identity stationary)
                nc.tensor.matmul(
                    ps[:, :], eye_sb[:, :], xg_tiles.pop(t)[:, :],
                    start=True, stop=(t == 0),
                )
                if t > 0:
                    for kc in range(8):
                        nc.tensor.matmul(
                            ps[:, :],
                            hsT[:, kc * NTOK + B * (t - 1) :
                                kc * NTOK + B * t],
                            wsb[5 + kc][:, :],
                            start=False,
                            stop=(kc == 7),
                        )

                gt = gates_p.tile([128, GS], F32, tag="gt")
                nc.scalar.activation(
                    gt[:, 0:384], ps[:, 0:384],
                    mybir.ActivationFunctionType.Sigmoid,
                )
                nc.scalar.activation(
                    gt[:, 384:512], ps[:, 384:512],
                    mybir.ActivationFunctionType.Tanh,
                )
                # c = f*c + i*g ; h = o*tanh(c)
                if t == 0:
                    nc.vector.tensor_mul(
                        c_sb[:, :], gt[:, 0:128], gt[:, 384:512]
                    )
                else:
                    tmp1 = ew_p.tile([128, HC], F32, tag="tmp1")
                    nc.vector.tensor_mul(tmp1[:, :], gt[:, 0:128], gt[:, 384:512])
                    nc.vector.tensor_mul(c_sb[:, :], gt[:, 128:256], c_sb[:, :])
                    nc.vector.tensor_add(c_sb[:, :], c_sb[:, :], tmp1[:, :])
                tcs = ew_p.tile([128, HC], F32, tag="tcs")
                nc.scalar.activation(
                    tcs[:, :], c_sb[:, :], mybir.ActivationFunctionType.Tanh
                )
                h_sb = ew_p.tile([128, HC], BF, tag="h_sb")
                nc.vector.tensor_mul(h_sb[:, :], gt[:, 256:384], tcs[:, :])

                # PE transpose h chunk -> [128 H-sub, 128 batch]
                tps = tps_p.tile([128, 128], BF, tag="tps")
                nc.tensor.transpose(tps[:, :], h_sb[:, :], eye_sb[:, :])
                hT = ht_p.tile([128, 128], BF, tag="hT")
                nc.vector.tensor_copy(hT[:, :], tps[:, :])

                # exchange: AllGather h^T chunks via DRAM bounce
                bi = bounce_in[t % 2]
                bo = bounce_out[t % 2]
                nc.sync.dma_start(bi[:, :], hT[:, :])
                nc.gpsimd.collective_compute(
                    "AllGather",
                    mybir.AluOpType.bypass,
                    replica_groups=[list(range(NCORES))],
                    ins=[bi[:, :].opt()],
                    outs=[bo[:, :, :].opt()],
                )
                # single strided read-back into the fused hsT tile, on the
                # otherwise-idle Pool queue (natural consumer of the CC;
                # keeps the CC-completion wait off the SP queue head).
                hview = hsT[:, :].rearrange("p (kc tok) -> p kc tok", kc=8)
                nc.gpsimd.dma_start(
                    out=hview[:, :, B * t : B * (t + 1)],
                    in_=bo[:, :, :].rearrange("kc p b -> p kc b"),
                )

            for v in range(NVT):
                fc_queue.append((t_steps // FC_BLOCK - 1, v))
            emit_fc(len(fc_queue))

    nc.finalize()
    return nc


def prep_host(features, captions, embed_W, W_ih, W_hh, b_ih, b_hh, fc_W, fc_b,
              t_steps=T):
    """Host-side layout prep. Returns (shared dict, per-core lists)."""
    W_cat = np.concatenate([W_ih, W_hh], axis=1)  # [4H, E+H]
    bias = (b_ih + b_hh).astype(np.float32)

    emb = np.concatenate(
        [embed_W.astype(np.float32), features.astype(np.float32)], axis=0
    ).astype(BF16)
    idx = np.zeros((B, t_steps), np.int32)
    idx[:, 0] = V + np.arange(B)
    if t_steps > 1:
        idx[:, 1:] = captions[:, 1:t_steps].astype(np.int32)

    eye = np.eye(128, dtype=np.float32).astype(BF16)
    onesrow = np.zeros((128, 128), np.float32)
    onesrow[0, :] = 1.0
    onesrow = onesrow.astype(BF16)

    fc_W_pad = np.zeros((VPAD, H), np.float32)
    fc_W_pad[:V] = fc_W
    fc_b_pad = np.zeros((VPAD,), np.float32)
    fc_b_pad[:V] = fc_b

    shared = {"embed": emb, "idx": idx, "eye128": eye, "onesrow": onesrow}

    per_core = []
    for c in range(NCORES):
        # gate rows for this core: i,f,o,g each 128 rows at offset 128c
        hc = np.arange(128 * c, 128 * c + 128)
        rows = np.concatenate([hc, H + hc, 3 * H + hc, 2 * H + hc])
        W_sel = W_cat[rows]  # [512, 1536]
        wmov = np.zeros((13, 128, GS), np.float32)
        for k in range(4):
            wmov[k] = W_sel[:, 128 * k : 128 * k + 128].T
        wmov[4, 0, :] = bias[rows]
        for k in range(8):
            wmov[5 + k] = W_sel[:, E + 128 * k : E + 128 * k + 128].T
        wmov = wmov.astype(BF16)

        vrows = slice(NVT * 128 * c, NVT * 128 * (c + 1))
        # stationary layout per (v, kc): [128 K-part, 128 vocab cols]
        fcw = np.zeros((NVT, 128, 8 * 128), np.float32)
        Wv = fc_W_pad[vrows].reshape(NVT, 128, 8, 128)  # [v, vocab, kc, k]
        for v in range(NVT):
            for kc in range(8):
                fcw[v, :, 128 * kc : 128 * kc + 128] = Wv[v, :, kc, :].T
        fcw = fcw.astype(BF16)
        fcb = np.ascontiguousarray(
            fc_b_pad[vrows].reshape(NVT, 128).T
        ).astype(np.float32)
        per_core.append({"wmov": wmov, "fcw": fcw, "fcb": fcb})
    return shared, per_core


_NC_CACHE = {}


def kernel(features, captions, embed_W, W_ih, W_hh, b_ih, b_hh, fc_W, fc_b):
    from concourse.bass_utils import run_bass_kernel_spmd

    features = np.asarray(features)
    captions = np.asarray(captions)
    embed_W = np.asarray(embed_W)
    W_ih = np.asarray(W_ih)
    W_hh = np.asarray(W_hh)
    b_ih = np.asarray(b_ih)
    b_hh = np.asarray(b_hh)
    fc_W = np.asarray(fc_W)
    fc_b = np.asarray(fc_b)

    if "nc" not in _NC_CACHE:
        _NC_CACHE["nc"] = build_nc()
    nc = _NC_CACHE["nc"]

    shared, per_core = prep_host(
        features, captions, embed_W, W_ih, W_hh, b_ih, b_hh, fc_W, fc_b
    )
    in_maps = [{**shared, **pc} for pc in per_core]
    res = run_bass_kernel_spmd(nc, in_maps, list(range(NCORES)))
    _NC_CACHE["last_results"] = res
    _NC_CACHE["last_in_maps"] = in_maps

    out = np.empty((B, T, V), np.float32)
    for c in range(NCORES):
        lg = res.results[c]["out_lg"]  # [NVT, 128, T*B]
        # vocab row = 1280c + 128v + p ; token col = t*B + b
        chunk = lg.reshape(NVT * 128, T, B).transpose(2, 1, 0)  # [B, T, 1280]
        v0 = NVT * 128 * c
        v1 = min(V, v0 + NVT * 128)
        if v1 > v0:
            out[:, :, v0:v1] = chunk[:, :, : v1 - v0]
    return out


# revision 11
# speedup vs baseline: 1.5125x; 1.0417x over previous
"""Trainium2 Bass kernel for nn_Decoder (LSTM decoder: embed -> LSTM -> vocab fc).

Sharding: 8-way tensor-parallel over the 4H gate dim (and vocab for fc).
Each core computes ALL 128 batch rows x its 512 gate columns (128 of each
of i,f,o,g), so every matmul runs with a full 128-wide stationary operand.
Per step the core's h^T chunk (PE-transposed) is AllGathered via a DRAM
bounce; the 8 gathered chunks land in one fused hsT tile with a single
strided DMA.  Embedding rows are gathered in a prologue and staged to
DRAM; xs^T chunks stream back through transposing DMAs, feeding an
in-loop xg GEMM a few steps ahead.  fc (vocab-sharded, 10x128 rows per
core) is interleaved into the PE/Act idle time under the gather latency.
"""

import sys

sys.path.insert(0, "/opt/trn_rl_repo")

import numpy as np
import ml_dtypes

import concourse.bass as bass
import concourse.bacc as bacc
import concourse.mybir as mybir
import concourse.tile as tile

BF16 = ml_dtypes.bfloat16

# Problem shapes
B, T, E, H, V = 128, 64, 512, 1024, 10000
NCORES = 8
GS = 512            # gate cols per core (128 of each of i,f,o,g)
HC = 128            # h cols per core
NVT = 10            # vocab tiles of 128 per core (80 total, V padded 10240)
VPAD = NCORES * NVT * 128
NTOK = T * B        # 8192 tokens
LOOKAHEAD = 4       # xg compute-ahead distance (steps)
FC_BLOCK = 4        # steps per fc token block (512 tokens)

F32 = mybir.dt.float32
BF = mybir.dt.bfloat16
I32 = mybir.dt.int32


def build_nc(t_steps=T):
    nc = bacc.Bacc(num_devices=NCORES)

    embed_d = nc.declare_dram_parameter("embed", [V + B, E], BF, isOutput=False)
    idx_d = nc.declare_dram_parameter("idx", [B, t_steps], I32, isOutput=False)
    # wmov: k 0..3 = W_ih^T chunks, k 4 = bias row, k 5..12 = W_hh^T chunks
    wmov_d = nc.declare_dram_parameter("wmov", [13, 128, GS], BF, isOutput=False)
    eye_d = nc.declare_dram_parameter("eye128", [128, 128], BF, isOutput=False)
    ones_d = nc.declare_dram_parameter("onesrow", [128, 128], BF, isOutput=False)
    fcw_d = nc.declare_dram_parameter("fcw", [NVT, 128, 8 * 128], BF, isOutput=False)
    fcb_d = nc.declare_dram_parameter("fcb", [128, NVT], F32, isOutput=False)
    out_d = nc.declare_dram_parameter("out_lg", [NVT, 128, t_steps * B], F32,
                                      isOutput=True)
    xsr_d = nc.dram_tensor("xsr_stage", [t_steps, B, E], BF)

    blk_tok = FC_BLOCK * B

    with tile.TileContext(nc) as tc:
        with (
            tc.tile_pool(name="persist", bufs=1) as pp,
            tc.tile_pool(name="xsraw", bufs=6) as xsraw_p,
            tc.tile_pool(name="xst", bufs=3) as xst_p,
            tc.tile_pool(name="xg", bufs=LOOKAHEAD + 2) as xg_p,
            tc.tile_pool(name="gates", bufs=3) as gates_p,
            tc.tile_pool(name="ew", bufs=4) as ew_p,
            tc.tile_pool(name="ht", bufs=3) as ht_p,
            tc.tile_pool(name="logit", bufs=4) as logit_p,
            tc.tile_pool(name="gpsum", bufs=2, space="PSUM") as gps_p,
            tc.tile_pool(name="xpsum", bufs=1, space="PSUM") as xps_p,
            tc.tile_pool(name="tpsum", bufs=2, space="PSUM") as tps_p,
            tc.tile_pool(name="fpsum", bufs=3, space="PSUM") as fps_p,
            tc.tile_pool(name="dram", bufs=4, space="DRAM") as dram_p,
        ):
            # ---- persistent tiles + prologue loads ----
            wsb = {}
            for k in range(13):
                w = pp.tile([128, GS], BF, tag=f"w_{k}", name=f"w_{k}")
                nc.sync.dma_start(out=w[:, :], in_=wmov_d[k, :, :])
                wsb[k] = w
            idx_sb = pp.tile([B, t_steps], I32, tag="idx")
            nc.sync.dma_start(out=idx_sb[:, :], in_=idx_d[:, :])
            eye_sb = pp.tile([128, 128], BF, tag="eye")
            nc.sync.dma_start(out=eye_sb[:, :], in_=eye_d[:, :])
            ones_sb = pp.tile([128, 128], BF, tag="ones")
            nc.sync.dma_start(out=ones_sb[:, :], in_=ones_d[:, :])
            fcw_sb = []
            for v in range(NVT):
                w = pp.tile([128, 8 * 128], BF, tag=f"fcw_{v}", name=f"fcw_{v}")
                nc.sync.dma_start(out=w[:, :], in_=fcw_d[v, :, :])
                fcw_sb.append(w)
            fcb_sb = pp.tile([128, NVT], F32, tag="fcb")
            nc.sync.dma_start(out=fcb_sb[:, :], in_=fcb_d[:, :])

            # fused h^T history: free index = kc*NTOK + t*B + b
            hsT = pp.tile([128, 8 * NTOK], BF, tag="hsT", name="hsT")
            c_sb = pp.tile([128, HC], F32, tag="c_state")

            # collective bounce buffers (DRAM)
            bounce_in = [dram_p.tile([128, 128], BF, name=f"bi_{i}")
                         for i in range(2)]
            bounce_out = [dram_p.tile([8, 128, 128], BF, name=f"bo_{i}")
                          for i in range(2)]

            # ---- prologue: gather embeddings, stage to DRAM ----
            for t in range(t_steps):
                xsr = xsraw_p.tile([B, E], BF, tag="xsr")
                nc.gpsimd.indirect_dma_start(
                    out=xsr[:, :],
                    out_offset=None,
                    in_=embed_d[:, :],
                    in_offset=bass.IndirectOffsetOnAxis(
                        ap=idx_sb[:, t : t + 1], axis=0
                    ),
                )
                nc.sync.dma_start(out=xsr_d[t, :, :], in_=xsr[:, :])

            # ---- xg pipeline: transposing-DMA xs^T chunks -> gemm ----
            xg_tiles = {}

            def make_xg(t):
                xst = xst_p.tile([128, 4 * 128], BF, tag="xst")
                for ec in range(4):
                    nc.sync.dma_start_transpose(
                        out=xst[:, 128 * ec : 128 * ec + 128],
                        in_=xsr_d[t, :, 128 * ec : 128 * ec + 128],
                    )
                xps = xps_p.tile([128, GS], F32, tag="xps")
                for ec in range(4):
                    nc.tensor.matmul(
                        xps[:, :],
                        xst[:, 128 * ec : 128 * ec + 128],
                        wsb[ec][:, :],
                        start=(ec == 0),
                        stop=False,
                    )
                nc.tensor.matmul(
                    xps[:, :], ones_sb[:, :], wsb[4][:, :], start=False, stop=True
                )
                xg = xg_p.tile([B, GS], BF, tag="xg")
                nc.scalar.activation(
                    xg[:, :], xps[:, :], mybir.ActivationFunctionType.Identity
                )
                xg_tiles[t] = xg

            for t in range(LOOKAHEAD):
                make_xg(t)

            # ---- fc emission helper ----
            fc_queue = []  # (block_idx, vtile)

            def emit_fc(n):
                for _ in range(min(n, len(fc_queue))):
                    kblk, v = fc_queue.pop(0)
                    fps = fps_p.tile([128, blk_tok], F32, tag="fps")
                    for kc in range(8):
                        nc.tensor.matmul(
                            fps[:, :],
                            fcw_sb[v][:, 128 * kc : 128 * kc + 128],
                            hsT[:, kc * NTOK + blk_tok * kblk :
                                kc * NTOK + blk_tok * (kblk + 1)],
                            start=(kc == 0),
                            stop=(kc == 7),
                        )
                    lg = logit_p.tile([128, blk_tok], F32, tag="lg")
                    nc.scalar.activation(
                        lg[:, :],
                        fps[:, :],
                        mybir.ActivationFunctionType.Identity,
                        bias=fcb_sb[:, v : v + 1],
                    )
                    nc.sync.dma_start(
                        out=out_d[v, :, blk_tok * kblk : blk_tok * (kblk + 1)],
                        in_=lg[:, :],
                    )

            # ---- recurrence ----
            for t in range(t_steps):
                # emit future-xg and fc work FIRST so their DMAs/mms sit
                # ahead of the exchange in every queue and fill the
                # collective-latency window (no head-of-line blocking).
                if t + LOOKAHEAD < t_steps:
                    make_xg(t + LOOKAHEAD)
                if t % FC_BLOCK == 0 and t >= FC_BLOCK:
                    kblk = t // FC_BLOCK - 1
                    for v in range(NVT):
                        fc_queue.append((kblk, v))
                if t >= FC_BLOCK:
                    emit_fc(3)

                ps = gps_p.tile([128, GS], F32, tag="gps")
                # xg injection (identity stationary)
                nc.tensor.matmul(
                    ps[:, :], eye_sb[:, :], xg_tiles.pop(t)[:, :],
                    start=True, stop=(t == 0),
                )
                if t > 0:
                    for kc in range(8):
                        nc.tensor.matmul(
                            ps[:, :],
                            hsT[:, kc * NTOK + B * (t - 1) :
                                kc * NTOK + B * t],
                            wsb[5 + kc][:, :],
                            start=False,
                            stop=(kc == 7),
                        )

                gt = gates_p.tile([128, GS], F32, tag="gt")
                nc.scalar.activation(
                    gt[:, 0:384], ps[:, 0:384],
                    mybir.ActivationFunctionType.Sigmoid,
                )
                nc.scalar.activation(
                    gt[:, 384:512], ps[:, 384:512],
                    mybir.ActivationFunctionType.Tanh,
                )
                # c = f*c + i*g ; h = o*tanh(c)
                if t == 0:
                    nc.vector.tensor_mul(
                        c_sb[:, :], gt[:, 0:128], gt[:, 384:512]
                    )
                else:
                    tmp1 = ew_p.tile([128, HC], F32, tag="tmp1")
                    # i*g on the (otherwise idle) Pool engine, f*c on DVE
                    nc.gpsimd.tensor_mul(tmp1[:, :], gt[:, 0:128], gt[:, 384:512])
                    nc.vector.tensor_mul(c_sb[:, :], gt[:, 128:256], c_sb[:, :])
                    nc.vector.tensor_add(c_sb[:, :], c_sb[:, :], tmp1[:, :])
                tcs = ew_p.tile([128, HC], F32, tag="tcs")
                nc.scalar.activation(
                    tcs[:, :], c_sb[:, :], mybir.ActivationFunctionType.Tanh
                )
                h_sb = ew_p.tile([128, HC], BF, tag="h_sb")
                nc.vector.tensor_mul(h_sb[:, :], gt[:, 256:384], tcs[:, :])

                # PE transpose h chunk -> [128 H-sub, 128 batch]
                tps = tps_p.tile([128, 128], BF, tag="tps")
                nc.tensor.transpose(tps[:, :], h_sb[:, :], eye_sb[:, :])
                hT = ht_p.tile([128, 128], BF, tag="hT")
                nc.scalar.copy(hT[:, :], tps[:, :])

                # exchange: AllGather h^T chunks via DRAM bounce
                # (evict + bounce-in DMA both on Act: one fewer sem hop)
                bi = bounce_in[t % 2]
                bo = bounce_out[t % 2]
                nc.scalar.dma_start(bi[:, :], hT[:, :])
                nc.gpsimd.collective_compute(
                    "AllGather",
                    mybir.AluOpType.bypass,
                    replica_groups=[list(range(NCORES))],
                    ins=[bi[:, :].opt()],
                    outs=[bo[:, :, :].opt()],
                )
                # single strided read-back into the fused hsT tile, on the
                # otherwise-idle Pool queue (natural consumer of the CC;
                # keeps the CC-completion wait off the SP queue head).
                hview = hsT[:, :].rearrange("p (kc tok) -> p kc tok", kc=8)
                nc.gpsimd.dma_start(
                    out=hview[:, :, B * t : B * (t + 1)],
                    in_=bo[:, :, :].rearrange("kc p b -> p kc b"),
                )

            for v in range(NVT):
                fc_queue.append((t_steps // FC_BLOCK - 1, v))
            emit_fc(len(fc_queue))

    nc.finalize()
    return nc


def prep_host(features, captions, embed_W, W_ih, W_hh, b_ih, b_hh, fc_W, fc_b,
              t_steps=T):
    """Host-side layout prep. Returns (shared dict, per-core lists)."""
    W_cat = np.concatenate([W_ih, W_hh], axis=1)  # [4H, E+H]
    bias = (b_ih + b_hh).astype(np.float32)

    emb = np.concatenate(
        [embed_W.astype(np.float32), features.astype(np.float32)], axis=0
    ).astype(BF16)
    idx = np.zeros((B, t_steps), np.int32)
    idx[:, 0] = V + np.arange(B)
    if t_steps > 1:
        idx[:, 1:] = captions[:, 1:t_steps].astype(np.int32)

    eye = np.eye(128, dtype=np.float32).astype(BF16)
    onesrow = np.zeros((128, 128), np.float32)
    onesrow[0, :] = 1.0
    onesrow = onesrow.astype(BF16)

    fc_W_pad = np.zeros((VPAD, H), np.float32)
    fc_W_pad[:V] = fc_W
    fc_b_pad = np.zeros((VPAD,), np.float32)
    fc_b_pad[:V] = fc_b

    shared = {"embed": emb, "idx": idx, "eye128": eye, "onesrow": onesrow}

    per_core = []
    for c in range(NCORES):
        # gate rows for this core: i,f,o,g each 128 rows at offset 128c
        hc = np.arange(128 * c, 128 * c + 128)
        rows = np.concatenate([hc, H + hc, 3 * H + hc, 2 * H + hc])
        W_sel = W_cat[rows]  # [512, 1536]
        wmov = np.zeros((13, 128, GS), np.float32)
        for k in range(4):
            wmov[k] = W_sel[:, 128 * k : 128 * k + 128].T
        wmov[4, 0, :] = bias[rows]
        for k in range(8):
            wmov[5 + k] = W_sel[:, E + 128 * k : E + 128 * k + 128].T
        wmov = wmov.astype(BF16)

        vrows = slice(NVT * 128 * c, NVT * 128 * (c + 1))
        # stationary layout per (v, kc): [128 K-part, 128 vocab cols]
        fcw = np.zeros((NVT, 128, 8 * 128), np.float32)
        Wv = fc_W_pad[vrows].reshape(NVT, 128, 8, 128)  # [v, vocab, kc, k]
        for v in range(NVT):
            for kc in range(8):
                fcw[v, :, 128 * kc : 128 * kc + 128] = Wv[v, :, kc, :].T
        fcw = fcw.astype(BF16)
        fcb = np.ascontiguousarray(
            fc_b_pad[vrows].reshape(NVT, 128).T
        ).astype(np.float32)
        per_core.append({"wmov": wmov, "fcw": fcw, "fcb": fcb})
    return shared, per_core


_NC_CACHE = {}


def kernel(features, captions, embed_W, W_ih, W_hh, b_ih, b_hh, fc_W, fc_b):
    from concourse.bass_utils import run_bass_kernel_spmd

    features = np.asarray(features)
    captions = np.asarray(captions)
    embed_W = np.asarray(embed_W)
    W_ih = np.asarray(W_ih)
    W_hh = np.asarray(W_hh)
    b_ih = np.asarray(b_ih)
    b_hh = np.asarray(b_hh)
    fc_W = np.asarray(fc_W)
    fc_b = np.asarray(fc_b)

    if "nc" not in _NC_CACHE:
        _NC_CACHE["nc"] = build_nc()
    nc = _NC_CACHE["nc"]

    shared, per_core = prep_host(
        features, captions, embed_W, W_ih, W_hh, b_ih, b_hh, fc_W, fc_b
    )
    in_maps = [{**shared, **pc} for pc in per_core]
    res = run_bass_kernel_spmd(nc, in_maps, list(range(NCORES)))
    _NC_CACHE["last_results"] = res
    _NC_CACHE["last_in_maps"] = in_maps

    out = np.empty((B, T, V), np.float32)
    for c in range(NCORES):
        lg = res.results[c]["out_lg"]  # [NVT, 128, T*B]
        # vocab row = 1280c + 128v + p ; token col = t*B + b
        chunk = lg.reshape(NVT * 128, T, B).transpose(2, 1, 0)  # [B, T, 1280]
        v0 = NVT * 128 * c
        v1 = min(V, v0 + NVT * 128)
        if v1 > v0:
            out[:, :, v0:v1] = chunk[:, :, : v1 - v0]
    return out
